# revision 29
# baseline (speedup 1.0000x reference)
"""Trainium2 Bass kernel for AcceleratedAttentionPool1d (v2).

Algebra: only the CENTER row of each window's attention survives, so per
output position s:
  qtok = (Wq @ xp + bq)/sqrt(24)            (scale folded into weights)
  energy[s, j] = <qtok[:, s+4], qtok[:, s0+j]>  over a 9-wide band
  attn = softmax(energy) over the band
  out[:, s] = (Wo/9) @ (sum_j attn[s,j] xp[:, s0+j]) + bo/9
The output projection folds into the V side: WXh[c][j, f] = sum_e
xp[e, s0+j]*(Wo/9)[f, e], so attn @ WXh is the final output directly.

Sharding: data-parallel over batch; B=8 batches on 8 cores.

v2 structure (vs v1):
 - bf16 matmul operands AND bf16 output (host converts to fp32):
   fp32r matmuls with free-dim <256 run at 4 cyc/row on the PE; bf16 is
   1 cyc/row everywhere. Measured rel err ~4e-3 vs the 2e-2 gate.
 - DMA: 9 input dma_starts (was 33), each a packed [128, bytes]
   partition-contiguous blob (128 descriptors), split across the sync/
   scalar/gpsimd rings so issue (~5ns/descriptor, serial per ring)
   overlaps. Output is a packed [128, 2, S] bf16 dram tensor the host
   unpacks; 4 output dma_starts on the idle sync ring.
 - Stage 1 (qtok+WXh) and the attention phase are INTERLEAVED by xp
   prefix arrival, so PE stays continuously busy (p-state: the PE runs
   2x slower unless busy; gaps reset it).
 - Softmax in [C, H] orientation, SOFT_G=3 chunks fused per PSUM bank.
 - Engine balance: qtok evict=vector(tensor_scalar_add bias), wxh
   evict=gpsimd, exp=scalar, reduce=gpsimd, recip+norm+mask=vector,
   at evict=gpsimd, fin evict=scalar(bias); A tiles persistent with
   one-time pad-row memset (no per-subgroup memsets).
 - PSUM: psq 2 + pswx 1 + pse 2 + psat 1 + psf 2 = 8 banks.
"""

import os
import numpy as np
import ml_dtypes

import concourse.bass as bass
import concourse.mybir as mybir
import concourse.tile as tile
from concourse import bacc
from concourse.bass import ts
from concourse.bass_utils import run_bass_kernel_spmd

F32 = mybir.dt.float32
BF16 = mybir.dt.bfloat16

B, E, S = 8, 256, 2048
KERNEL = 9
PAD = KERNEL // 2
SP = S + 2 * PAD  # 2056
C = 120  # output positions per chunk
H = 128  # halo width
NCHUNK = 18  # 17 full strides + 1 overlapping tail chunk
SOFT_G = 3  # chunks per fused softmax subgroup
NSG = NCHUNK // SOFT_G
GROUP = 6  # chunks per output tile/store
NEG = -1.0e30
TA, TB = 1024, SP - 1024  # xp dma split

T_CH = [(0, 512), (512, 512), (1024, 512), (1536, 512), (2048, 8)]


def _cs(c: int) -> int:
    return 120 * c if c < NCHUNK - 1 else S - C  # last chunk overlaps


def build_nc() -> bass.Bass:
    nc = bacc.Bacc("TRN2", target_bir_lowering=False)

    x0a_d = nc.dram_tensor("x0a", [128, TA], BF16, kind="ExternalInput")
    x0b_d = nc.dram_tensor("x0b", [128, TB], BF16, kind="ExternalInput")
    x1a_d = nc.dram_tensor("x1a", [128, TA], BF16, kind="ExternalInput")
    x1b_d = nc.dram_tensor("x1b", [128, TB], BF16, kind="ExternalInput")
    wqt_d = nc.dram_tensor("wqt", [128, 2, E], BF16, kind="ExternalInput")
    wot_d = nc.dram_tensor("wot", [128, 2, E], BF16, kind="ExternalInput")
    mask_d = nc.dram_tensor("maskd", [128, H], F32, kind="ExternalInput")
    id_d = nc.dram_tensor("identd", [128, 128], BF16, kind="ExternalInput")
    bq_d = nc.dram_tensor("bqvd", [128, 2], F32, kind="ExternalInput")
    bo_d = nc.dram_tensor("bovd", [128, 2], F32, kind="ExternalInput")
    out_d = nc.dram_tensor("out", [128, 2, S], BF16, kind="ExternalOutput")

    with tile.TileContext(nc) as tc:
        with (
            tc.tile_pool(name="const", bufs=1) as const,
            tc.tile_pool(name="work", bufs=4) as work,
            tc.tile_pool(name="grp", bufs=2) as grp,
            tc.tile_pool(name="ps", bufs=1, space="PSUM") as ps,
        ):
            # ---- DMA intake: sync ring carries weights+x (in consumption
            # order); scalar/gpsimd rings carry the small consts in parallel.
            wqt_t = const.tile([128, 2, E], BF16)
            nc.sync.dma_start(wqt_t, wqt_d[:, :, :])
            x0_t = const.tile([128, SP], BF16)
            x1_t = const.tile([128, SP], BF16)
            nc.sync.dma_start(x0_t[:, 0:TA], x0a_d[:, :])
            nc.sync.dma_start(x1_t[:, 0:TA], x1a_d[:, :])
            bq_t = const.tile([128, 2], F32)
            nc.scalar.dma_start(bq_t, bq_d[:, :])
            wot_t = const.tile([128, 2, E], BF16)
            nc.scalar.dma_start(wot_t, wot_d[:, :, :])
            # x second halves issue from the scalar ring after wot: they
            # overlap the sync ring's issue serialization but execute late
            # enough not to contend with the first halves the PE waits on
            nc.scalar.dma_start(x0_t[:, TA:SP], x0b_d[:, :])
            nc.scalar.dma_start(x1_t[:, TA:SP], x1b_d[:, :])
            bo_t = const.tile([128, 2], F32)
            nc.scalar.dma_start(bo_t, bo_d[:, :])

            mask_t = const.tile([128, H], F32)
            nc.gpsimd.dma_start(mask_t, mask_d[:, :])
            id_t = const.tile([128, 128], BF16)
            nc.gpsimd.dma_start(id_t, id_d[:, :])

            xs = [x0_t, x1_t]
            qtok0 = const.tile([128, SP], BF16)
            qtok1 = const.tile([128, SP], BF16)
            qtoks = [qtok0, qtok1]
            wxh_t = const.tile([128, NCHUNK, E], BF16)

            # persistent attention tiles; pad rows zeroed once
            A0 = const.tile([128, SOFT_G, H], BF16)
            A1 = const.tile([128, SOFT_G, H], BF16)
            nc.gpsimd.memset(A0[96:128, :, :].bitcast(mybir.dt.uint32), 0)
            nc.gpsimd.memset(A1[96:128, :, :].bitcast(mybir.dt.uint32), 0)
            A_slots = [A0, A1]

            out_view = out_d[:, :, :]
            fo_tiles = {}
            pf_state = {}
            state = {"wx": 0, "sg": 0, "pwx": None}

            def emit_wxh_upto(prefix):
                while state["wx"] < NCHUNK and _cs(state["wx"]) + H <= prefix:
                    c = state["wx"]
                    ci = c % 2
                    if ci == 0:
                        state["pwx"] = ps.tile(
                            [128, 2, E], F32, tag="pswx", bufs=1, name="pwx"
                        )
                    pwx = state["pwx"]
                    s0 = _cs(c)
                    for e_i in range(2):
                        nc.tensor.matmul(
                            pwx[:, ci, :],
                            lhsT=xs[e_i][:, s0 : s0 + H],
                            rhs=wot_t[:, e_i, :],
                            start=(e_i == 0),
                            stop=(e_i == 1),
                        )
                    if ci == 1:
                        nc.scalar.copy(wxh_t[:, c - 1 : c + 1, :], pwx)
                    state["wx"] += 1

            def emit_fin(c, at_ap):
                g = c // GROUP
                p = (c % GROUP) // 2
                ci = c % 2
                if ci == 0:
                    pf_state[(g, p)] = ps.tile(
                        [128, 2, 2, C], F32, tag="psf", bufs=2, name="pf"
                    )
                pf = pf_state[(g, p)]
                for f_i in range(2):
                    nc.tensor.matmul(
                        pf[:, f_i, ci, :],
                        lhsT=wxh_t[:, c, ts(f_i, 128)],
                        rhs=at_ap[:, 0:C],
                        start=True,
                        stop=True,
                    )
                if ci == 1:
                    fo = fo_tiles[g]
                    for f_i in range(2):
                        nc.scalar.activation(
                            fo[:, f_i, 240 * p : 240 * p + 240],
                            pf[:, f_i, :, :],
                            mybir.ActivationFunctionType.Identity,
                            bias=bo_t[:, f_i : f_i + 1],
                            scale=1.0,
                        )
                    # last group stores per pair so the final drain is short
                    if g == 2:
                        if p < 2:
                            nc.sync.dma_start(
                                out_view[:, :, 1440 + 240 * p : 1680 + 240 * p],
                                fo[:, :, 240 * p : 240 * p + 240],
                            )
                        else:
                            nc.sync.dma_start(
                                out_view[:, :, 1920:2040], fo[:, :, 480:600]
                            )
                            nc.sync.dma_start(
                                out_view[:, :, 2040:2048], fo[:, :, 712:720]
                            )

            def emit_sg(k):
                g = k // 2
                if k % 2 == 0:
                    fo_tiles[g] = grp.tile(
                        [128, 2, GROUP * C], BF16, tag="fo", name="fo"
                    )
                pe_ = ps.tile([C, SOFT_G, H], F32, tag="pse", bufs=2, name="pe_")
                for gi in range(SOFT_G):
                    c = SOFT_G * k + gi
                    s0 = _cs(c)
                    for f_i in range(2):
                        nc.tensor.matmul(
                            pe_[:, gi, :],
                            lhsT=qtoks[f_i][:, s0 + PAD : s0 + PAD + C],
                            rhs=qtoks[f_i][:, s0 : s0 + H],
                            start=(f_i == 0),
                            stop=(f_i == 1),
                        )
                nc.vector.tensor_tensor(
                    out=pe_,
                    in0=pe_,
                    in1=mask_t[:C, None, :].to_broadcast((C, SOFT_G, H)),
                    op=mybir.AluOpType.add,
                )
                A = A_slots[k % 2]
                nc.scalar.activation(
                    A[:C, :, :], pe_, mybir.ActivationFunctionType.Exp
                )
                sums = work.tile([C, SOFT_G], F32, tag="sums", name="sums")
                nc.vector.tensor_reduce(
                    sums,
                    A[:C, :, :],
                    axis=mybir.AxisListType.X,
                    op=mybir.AluOpType.add,
                )
                r = work.tile([C, SOFT_G], F32, tag="r", name="r")
                nc.vector.reciprocal(r, sums)
                nc.vector.tensor_tensor(
                    out=A[:C, :, :],
                    in0=A[:C, :, :],
                    in1=r[:, :, None].to_broadcast((C, SOFT_G, H)),
                    op=mybir.AluOpType.mult,
                )
                pat = ps.tile([128, SOFT_G, 128], BF16, tag="psat", bufs=1, name="pat")
                for gi in range(SOFT_G):
                    nc.tensor.transpose(pat[:, gi, :], A[:, gi, :], id_t)
                at = work.tile([128, SOFT_G, 128], BF16, tag="at", name="at")
                nc.vector.tensor_copy(at, pat)
                for gi in range(SOFT_G):
                    emit_fin(SOFT_G * k + gi, at[:, gi, :])
                if k % 2 == 1 and g < 2:
                    nc.sync.dma_start(
                        out_view[:, :, 720 * g : 720 * g + 720], fo_tiles[g]
                    )

            for t0, w in T_CH:
                for f_i in range(2):
                    pq = ps.tile([128, 512], F32, tag="psq", bufs=2, name="pq")
                    for e_i in range(2):
                        nc.tensor.matmul(
                            pq[:, :w],
                            lhsT=wqt_t[:, e_i, ts(f_i, 128)],
                            rhs=xs[e_i][:, t0 : t0 + w],
                            start=(e_i == 0),
                            stop=(e_i == 1),
                        )
                    # split the two evictions across engines so they run
                    # concurrently — every subgroup's energy matmuls wait on
                    # both, and scalar has slack this early in the pipeline
                    if f_i == 0:
                        nc.vector.tensor_scalar_add(
                            qtoks[0][:, t0 : t0 + w], pq[:, :w], bq_t[:, 0:1]
                        )
                    else:
                        nc.scalar.activation(
                            qtoks[1][:, t0 : t0 + w],
                            pq[:, :w],
                            mybir.ActivationFunctionType.Identity,
                            bias=bq_t[:, 1:2],
                            scale=1.0,
                        )
                prefix = t0 + w
                emit_wxh_upto(prefix)
                while (
                    state["sg"] < NSG
                    and _cs(SOFT_G * state["sg"] + 2) + H <= prefix
                ):
                    emit_sg(state["sg"])
                    state["sg"] += 1
            while state["sg"] < NSG:
                emit_sg(state["sg"])
                state["sg"] += 1

    nc.compile()
    return nc


def make_in_maps(x, Wq, bq, Wo, bo):
    x = np.asarray(x, dtype=np.float32)
    Wq = np.asarray(Wq, dtype=np.float32)
    bq = np.asarray(bq, dtype=np.float32)
    Wo = np.asarray(Wo, dtype=np.float32)
    bo = np.asarray(bo, dtype=np.float32)

    bf = ml_dtypes.bfloat16
    f = 1.0 / np.sqrt(np.sqrt(E) * 1.5)  # 1/sqrt(24) folded into Wq, bq
    wqt = np.ascontiguousarray(
        (Wq * f).T.reshape(2, 128, E).transpose(1, 0, 2)
    ).astype(bf)
    wot = np.ascontiguousarray(
        (Wo / KERNEL).T.reshape(2, 128, E).transpose(1, 0, 2)
    ).astype(bf)
    bqv = np.ascontiguousarray((bq * f).reshape(2, 128).T)
    bov = np.ascontiguousarray((bo / KERNEL).reshape(2, 128).T)

    mask = np.full((128, H), NEG, dtype=np.float32)
    for m in range(128):
        mask[m, m : min(m + KERNEL, H)] = 0.0
    ident = np.eye(128, dtype=bf)

    in_maps = []
    for b in range(B):
        xp = np.zeros((E, SP), dtype=np.float32)
        xp[:, PAD : PAD + S] = x[b]
        xpb = xp.astype(bf)
        in_maps.append(
            dict(
                x0a=np.ascontiguousarray(xpb[0:128, 0:TA]),
                x0b=np.ascontiguousarray(xpb[0:128, TA:SP]),
                x1a=np.ascontiguousarray(xpb[128:256, 0:TA]),
                x1b=np.ascontiguousarray(xpb[128:256, TA:SP]),
                wqt=wqt,
                wot=wot,
                maskd=mask,
                identd=ident,
                bqvd=bqv,
                bovd=bov,
            )
        )
    return in_maps


_NC_CACHE = {}


def kernel(x, Wq, bq, Wo, bo):
    res = kernel_with_results(x, Wq, bq, Wo, bo)
    outs = []
    for r in res.results:
        o = np.asarray(r["out"])  # [128, 2, S] bf16
        outs.append(o.transpose(1, 0, 2).reshape(E, S).astype(np.float32))
    return np.stack(outs)


def kernel_with_results(x, Wq, bq, Wo, bo, trace=False, **kwargs):
    in_maps = make_in_maps(x, Wq, bq, Wo, bo)
    if "nc" not in _NC_CACHE:
        _NC_CACHE["nc"] = build_nc()
    return run_bass_kernel_spmd(
        _NC_CACHE["nc"], in_maps, core_ids=list(range(B)), trace=trace, **kwargs
    )


# revision 30
# speedup vs baseline: 1.1288x; 1.1288x over previous
"""Trainium2 Bass kernel for AcceleratedAttentionPool1d (v2).

Algebra: only the CENTER row of each window's attention survives, so per
output position s:
  qtok = (Wq @ xp + bq)/sqrt(24)            (scale folded into weights)
  energy[s, j] = <qtok[:, s+4], qtok[:, s0+j]>  over a 9-wide band
  attn = softmax(energy) over the band
  out[:, s] = (Wo/9) @ (sum_j attn[s,j] xp[:, s0+j]) + bo/9
The output projection folds into the V side: WXh[c][j, f] = sum_e
xp[e, s0+j]*(Wo/9)[f, e], so attn @ WXh is the final output directly.

Sharding: data-parallel over batch; B=8 batches on 8 cores.

v2 structure (vs v1):
 - bf16 matmul operands AND bf16 output (host converts to fp32):
   fp32r matmuls with free-dim <256 run at 4 cyc/row on the PE; bf16 is
   1 cyc/row everywhere. Measured rel err ~4e-3 vs the 2e-2 gate.
 - DMA: 9 input dma_starts (was 33), each a packed [128, bytes]
   partition-contiguous blob (128 descriptors), split across the sync/
   scalar/gpsimd rings so issue (~5ns/descriptor, serial per ring)
   overlaps. Output is a packed [128, 2, S] bf16 dram tensor the host
   unpacks; 4 output dma_starts on the idle sync ring.
 - Stage 1 (qtok+WXh) and the attention phase are INTERLEAVED by xp
   prefix arrival, so PE stays continuously busy (p-state: the PE runs
   2x slower unless busy; gaps reset it).
 - Softmax in [C, H] orientation, SOFT_G=3 chunks fused per PSUM bank.
 - Engine balance: qtok evict=vector(tensor_scalar_add bias), wxh
   evict=gpsimd, exp=scalar, reduce=gpsimd, recip+norm+mask=vector,
   at evict=gpsimd, fin evict=scalar(bias); A tiles persistent with
   one-time pad-row memset (no per-subgroup memsets).
 - PSUM: psq 2 + pswx 1 + pse 2 + psat 1 + psf 2 = 8 banks.
"""

import os
import numpy as np
import ml_dtypes

import concourse.bass as bass
import concourse.mybir as mybir
import concourse.tile as tile
from concourse import bacc
from concourse.bass import ts
from concourse.bass_utils import run_bass_kernel_spmd

F32 = mybir.dt.float32
BF16 = mybir.dt.bfloat16

B, E, S = 8, 256, 2048
KERNEL = 9
PAD = KERNEL // 2
SP = S + 2 * PAD  # 2056
C = 120  # output positions per chunk
H = 128  # halo width
NCHUNK = 18  # 17 full strides + 1 overlapping tail chunk
SOFT_G = 3  # chunks per fused softmax subgroup
NSG = NCHUNK // SOFT_G
GROUP = 6  # chunks per output tile/store
NEG = -1.0e30
TA, TB = 1024, SP - 1024  # xp dma split

T_CH = [(0, 512), (512, 512), (1024, 512), (1536, 512), (2048, 8)]


def _cs(c: int) -> int:
    return 120 * c if c < NCHUNK - 1 else S - C  # last chunk overlaps


def build_nc() -> bass.Bass:
    nc = bacc.Bacc("TRN2", target_bir_lowering=False)

    x0a_d = nc.dram_tensor("x0a", [128, TA], BF16, kind="ExternalInput")
    x0b_d = nc.dram_tensor("x0b", [128, TB], BF16, kind="ExternalInput")
    x1a_d = nc.dram_tensor("x1a", [128, TA], BF16, kind="ExternalInput")
    x1b_d = nc.dram_tensor("x1b", [128, TB], BF16, kind="ExternalInput")
    wqt_d = nc.dram_tensor("wqt", [128, 2, E], BF16, kind="ExternalInput")
    wot_d = nc.dram_tensor("wot", [128, 2, E], BF16, kind="ExternalInput")
    mask_d = nc.dram_tensor("maskd", [128, H], F32, kind="ExternalInput")
    id_d = nc.dram_tensor("identd", [128, 128], BF16, kind="ExternalInput")
    bq_d = nc.dram_tensor("bqvd", [128, 2], F32, kind="ExternalInput")
    bo_d = nc.dram_tensor("bovd", [128, 2], F32, kind="ExternalInput")
    out_d = nc.dram_tensor("out", [128, 2, S], BF16, kind="ExternalOutput")

    with tile.TileContext(nc) as tc:
        with (
            tc.tile_pool(name="const", bufs=1) as const,
            tc.tile_pool(name="work", bufs=4) as work,
            tc.tile_pool(name="grp", bufs=2) as grp,
            tc.tile_pool(name="ps", bufs=1, space="PSUM") as ps,
        ):
            # ---- DMA intake: sync ring carries weights+x (in consumption
            # order); scalar/gpsimd rings carry the small consts in parallel.
            wqt_t = const.tile([128, 2, E], BF16)
            nc.sync.dma_start(wqt_t, wqt_d[:, :, :])
            x0_t = const.tile([128, SP], BF16)
            x1_t = const.tile([128, SP], BF16)
            nc.sync.dma_start(x0_t[:, 0:TA], x0a_d[:, :])
            nc.sync.dma_start(x1_t[:, 0:TA], x1a_d[:, :])
            bq_t = const.tile([128, 2], F32)
            nc.scalar.dma_start(bq_t, bq_d[:, :])
            wot_t = const.tile([128, 2, E], BF16)
            nc.scalar.dma_start(wot_t, wot_d[:, :, :])
            # x second halves issue from the scalar ring after wot: they
            # overlap the sync ring's issue serialization but execute late
            # enough not to contend with the first halves the PE waits on
            nc.scalar.dma_start(x0_t[:, TA:SP], x0b_d[:, :])
            nc.scalar.dma_start(x1_t[:, TA:SP], x1b_d[:, :])
            bo_t = const.tile([128, 2], F32)
            nc.scalar.dma_start(bo_t, bo_d[:, :])

            mask_t = const.tile([128, H], F32)
            nc.gpsimd.dma_start(mask_t, mask_d[:, :])
            id_t = const.tile([128, 128], BF16)
            nc.gpsimd.dma_start(id_t, id_d[:, :])

            xs = [x0_t, x1_t]
            qtok0 = const.tile([128, SP], BF16)
            qtok1 = const.tile([128, SP], BF16)
            qtoks = [qtok0, qtok1]
            wxh_t = const.tile([128, NCHUNK, E], BF16)

            # persistent attention tiles; pad rows zeroed once
            A0 = const.tile([128, SOFT_G, H], BF16)
            A1 = const.tile([128, SOFT_G, H], BF16)
            nc.gpsimd.memset(A0[96:128, :, :].bitcast(mybir.dt.uint32), 0)
            nc.gpsimd.memset(A1[96:128, :, :].bitcast(mybir.dt.uint32), 0)
            A_slots = [A0, A1]

            out_view = out_d[:, :, :]
            fo_tiles = {}
            pf_state = {}
            state = {"wx": 0, "sg": 0, "pwx": None}

            def emit_wxh_upto(prefix):
                while state["wx"] < NCHUNK and _cs(state["wx"]) + H <= prefix:
                    c = state["wx"]
                    ci = c % 2
                    if ci == 0:
                        state["pwx"] = ps.tile(
                            [128, 2, E], F32, tag="pswx", bufs=1, name="pwx"
                        )
                    pwx = state["pwx"]
                    s0 = _cs(c)
                    for e_i in range(2):
                        nc.tensor.matmul(
                            pwx[:, ci, :],
                            lhsT=xs[e_i][:, s0 : s0 + H],
                            rhs=wot_t[:, e_i, :],
                            start=(e_i == 0),
                            stop=(e_i == 1),
                        )
                    if ci == 1:
                        nc.scalar.copy(wxh_t[:, c - 1 : c + 1, :], pwx)
                    state["wx"] += 1

            def emit_fin(c, at_ap):
                g = c // GROUP
                p = (c % GROUP) // 2
                ci = c % 2
                if ci == 0:
                    pf_state[(g, p)] = ps.tile(
                        [128, 2, 2, C], F32, tag="psf", bufs=2, name="pf"
                    )
                pf = pf_state[(g, p)]
                for f_i in range(2):
                    nc.tensor.matmul(
                        pf[:, f_i, ci, :],
                        lhsT=wxh_t[:, c, ts(f_i, 128)],
                        rhs=at_ap[:, 0:C],
                        start=True,
                        stop=True,
                    )
                if ci == 1:
                    fo = fo_tiles[g]
                    for f_i in range(2):
                        nc.scalar.activation(
                            fo[:, f_i, 240 * p : 240 * p + 240],
                            pf[:, f_i, :, :],
                            mybir.ActivationFunctionType.Identity,
                            bias=bo_t[:, f_i : f_i + 1],
                            scale=1.0,
                        )
                    # last group stores per pair so the final drain is short
                    if g == 2:
                        if p < 2:
                            nc.sync.dma_start(
                                out_view[:, :, 1440 + 240 * p : 1680 + 240 * p],
                                fo[:, :, 240 * p : 240 * p + 240],
                            )
                        else:
                            nc.sync.dma_start(
                                out_view[:, :, 1920:2040], fo[:, :, 480:600]
                            )
                            nc.sync.dma_start(
                                out_view[:, :, 2040:2048], fo[:, :, 712:720]
                            )

            def emit_sg(k):
                g = k // 2
                if k % 2 == 0:
                    fo_tiles[g] = grp.tile(
                        [128, 2, GROUP * C], BF16, tag="fo", name="fo"
                    )
                pe_ = ps.tile([C, SOFT_G, H], F32, tag="pse", bufs=2, name="pe_")
                for gi in range(SOFT_G):
                    c = SOFT_G * k + gi
                    s0 = _cs(c)
                    for f_i in range(2):
                        nc.tensor.matmul(
                            pe_[:, gi, :],
                            lhsT=qtoks[f_i][:, s0 + PAD : s0 + PAD + C],
                            rhs=qtoks[f_i][:, s0 : s0 + H],
                            start=(f_i == 0),
                            stop=(f_i == 1),
                        )
                nc.vector.tensor_tensor(
                    out=pe_,
                    in0=pe_,
                    in1=mask_t[:C, None, :].to_broadcast((C, SOFT_G, H)),
                    op=mybir.AluOpType.add,
                )
                A = A_slots[k % 2]
                nc.scalar.activation(
                    A[:C, :, :], pe_, mybir.ActivationFunctionType.Exp
                )
                sums = work.tile([C, SOFT_G], F32, tag="sums", name="sums")
                nc.vector.tensor_reduce(
                    sums,
                    A[:C, :, :],
                    axis=mybir.AxisListType.X,
                    op=mybir.AluOpType.add,
                )
                r = work.tile([C, SOFT_G], F32, tag="r", name="r")
                nc.vector.reciprocal(r, sums)
                nc.vector.tensor_tensor(
                    out=A[:C, :, :],
                    in0=A[:C, :, :],
                    in1=r[:, :, None].to_broadcast((C, SOFT_G, H)),
                    op=mybir.AluOpType.mult,
                )
                pat = ps.tile([128, SOFT_G, 128], BF16, tag="psat", bufs=1, name="pat")
                for gi in range(SOFT_G):
                    nc.tensor.transpose(pat[:, gi, :], A[:, gi, :], id_t)
                at = work.tile([128, SOFT_G, 128], BF16, tag="at", name="at")
                nc.vector.tensor_copy(at, pat)
                for gi in range(SOFT_G):
                    emit_fin(SOFT_G * k + gi, at[:, gi, :])
                if k % 2 == 1 and g < 2:
                    nc.sync.dma_start(
                        out_view[:, :, 720 * g : 720 * g + 720], fo_tiles[g]
                    )

            for t0, w in T_CH:
                for f_i in range(2):
                    pq = ps.tile([128, 512], F32, tag="psq", bufs=2, name="pq")
                    for e_i in range(2):
                        nc.tensor.matmul(
                            pq[:, :w],
                            lhsT=wqt_t[:, e_i, ts(f_i, 128)],
                            rhs=xs[e_i][:, t0 : t0 + w],
                            start=(e_i == 0),
                            stop=(e_i == 1),
                        )
                    nc.vector.tensor_scalar_add(
                        qtoks[f_i][:, t0 : t0 + w], pq[:, :w], bq_t[:, f_i : f_i + 1]
                    )
                prefix = t0 + w
                emit_wxh_upto(prefix)
                while (
                    state["sg"] < NSG
                    and _cs(SOFT_G * state["sg"] + 2) + H <= prefix
                ):
                    emit_sg(state["sg"])
                    state["sg"] += 1
            while state["sg"] < NSG:
                emit_sg(state["sg"])
                state["sg"] += 1

    nc.compile()
    return nc


def make_in_maps(x, Wq, bq, Wo, bo):
    x = np.asarray(x, dtype=np.float32)
    Wq = np.asarray(Wq, dtype=np.float32)
    bq = np.asarray(bq, dtype=np.float32)
    Wo = np.asarray(Wo, dtype=np.float32)
    bo = np.asarray(bo, dtype=np.float32)

    bf = ml_dtypes.bfloat16
    f = 1.0 / np.sqrt(np.sqrt(E) * 1.5)  # 1/sqrt(24) folded into Wq, bq
    wqt = np.ascontiguousarray(
        (Wq * f).T.reshape(2, 128, E).transpose(1, 0, 2)
    ).astype(bf)
    wot = np.ascontiguousarray(
        (Wo / KERNEL).T.reshape(2, 128, E).transpose(1, 0, 2)
    ).astype(bf)
    bqv = np.ascontiguousarray((bq * f).reshape(2, 128).T)
    bov = np.ascontiguousarray((bo / KERNEL).reshape(2, 128).T)

    mask = np.full((128, H), NEG, dtype=np.float32)
    for m in range(128):
        mask[m, m : min(m + KERNEL, H)] = 0.0
    ident = np.eye(128, dtype=bf)

    in_maps = []
    for b in range(B):
        xp = np.zeros((E, SP), dtype=np.float32)
        xp[:, PAD : PAD + S] = x[b]
        xpb = xp.astype(bf)
        in_maps.append(
            dict(
                x0a=np.ascontiguousarray(xpb[0:128, 0:TA]),
                x0b=np.ascontiguousarray(xpb[0:128, TA:SP]),
                x1a=np.ascontiguousarray(xpb[128:256, 0:TA]),
                x1b=np.ascontiguousarray(xpb[128:256, TA:SP]),
                wqt=wqt,
                wot=wot,
                maskd=mask,
                identd=ident,
                bqvd=bqv,
                bovd=bov,
            )
        )
    return in_maps


_NC_CACHE = {}


def kernel(x, Wq, bq, Wo, bo):
    res = kernel_with_results(x, Wq, bq, Wo, bo)
    outs = []
    for r in res.results:
        o = np.asarray(r["out"])  # [128, 2, S] bf16
        outs.append(o.transpose(1, 0, 2).reshape(E, S).astype(np.float32))
    return np.stack(outs)


def kernel_with_results(x, Wq, bq, Wo, bo, trace=False, **kwargs):
    in_maps = make_in_maps(x, Wq, bq, Wo, bo)
    if "nc" not in _NC_CACHE:
        _NC_CACHE["nc"] = build_nc()
    return run_bass_kernel_spmd(
        _NC_CACHE["nc"], in_maps, core_ids=list(range(B)), trace=trace, **kwargs
    )
